# revision 32
# baseline (speedup 1.0000x reference)
"""MoE layer (E=8 experts, top-2, swiGLU) on 8 TRN2 NeuronCores.

Strategy: balanced expert-block dispatch. The router runs on host; each
core is assigned a fixed pattern of token blocks (same block sizes on
every core -> one SPMD program), and each block is bound to ONE expert
whose weights are streamed per block from per-core DRAM data. Packing
experts into the 8x[4,4,4,5]-tile block grid balances the padded token
count to C = ceil(sum_e ceil(count_e/128) / 8)*128 per core instead of
max_e count (2176 vs 2304 for the balanced-random router here).

All matmul operands are bf16 (PE full rate, FWL weight loads hide under
the matmul stream, half the DMA bytes); accumulation stays fp32 in
PSUM, swiGLU runs fp32 on ACT (fused Silu) + DVE, hT is stored bf16,
outputs are fp32. Measured end-to-end error ~4e-3 of absmax (gate 2e-2).

Schedule notes (from trace iterations):
- Block 0's tokens + first W1 tile are the only startup-critical bytes:
  they ride the sync HWDGE queue alone; every bulk load (other blocks'
  tokens, W2, gates) rides SWDGE anchored behind real matmuls so it
  cannot steal HBM bandwidth from the critical path at t=0.
- PSUM: six 1-bank tags rotate through the per-chunk psa/psb
  accumulators (reuse distance 3 chunks ~ 10us), so the ACT/DVE swiGLU
  chain never back-pressures the PE via PSUM WAR.
- Before each block's GEMM2 (which must wait for that block's LAST
  swiGLU write into hT), the NEXT block's first GEMM1 hidden tile is
  emitted as a filler so the PE has independent work during the
  hT-finalize latency.
- W1 tiles of block bi+1 are prefetched on the sync queue BEFORE block
  bi's y DMAs join that FIFO (w1 pool bufs=6 makes the slots free early).
"""

import math

import numpy as np
import ml_dtypes

import concourse.bacc as bacc
import concourse.bass as bass  # noqa: F401
import concourse.mybir as mybir
import concourse.tile as tile
from concourse.bass_utils import run_bass_kernel_spmd
from concourse.tile import add_dep_helper

P = 128
NCORES = 8

f32 = mybir.dt.float32
bf16 = mybir.dt.bfloat16
np_bf16 = ml_dtypes.bfloat16
SILU = mybir.ActivationFunctionType.Silu


def _chunks(tw, step=512):
    out = []
    c0 = 0
    while c0 < tw:
        out.append((c0, min(step, tw - c0)))
        c0 += step
    return out


def build_moe_nc(D, H, TWs, has_b1=False):
    """One SPMD program: len(TWs) token blocks, sizes TWs (multiples of
    128), each block bound to its own W1/W2 slice of the per-core weight
    stream tensors."""
    KO1 = D // P       # GEMM1 contraction tiles (over D)
    MP = H // P        # hidden tiles (per swiGLU half)
    KO2 = H // P       # GEMM2 contraction tiles (over H)
    NB = len(TWs)
    C = sum(TWs)
    n2chunks = _chunks(D)

    nc = bacc.Bacc(None)
    xt_d = nc.declare_dram_parameter("xt", [P, KO1, C], bf16, isOutput=False)
    w1_d = nc.declare_dram_parameter(
        "w1", [NB, MP, P, 2, KO1, P], bf16, isOutput=False
    )
    w2_d = nc.declare_dram_parameter("w2", [NB, P, KO2, D], bf16, isOutput=False)
    g_d = nc.declare_dram_parameter("g", [P, C // P], f32, isOutput=False)
    if has_b1:
        b1_d = nc.declare_dram_parameter("b1", [P, NB, 2, MP], f32, isOutput=False)
    y_d = nc.declare_dram_parameter("y", [C, D], f32, isOutput=True)

    W1_PREFETCH = 3

    with tile.TileContext(nc) as tc:
        with (
            tc.tile_pool(name="const", bufs=1) as const,
            tc.tile_pool(name="w1p", bufs=6) as w1p,
            tc.tile_pool(name="w2p", bufs=2) as w2p,
            tc.tile_pool(name="ev", bufs=2) as ev,
            tc.tile_pool(name="ps1", bufs=1, space="PSUM") as ps1,
            tc.tile_pool(name="ps2", bufs=2, space="PSUM") as ps2,
        ):
            # PE warmup: the HAM clock gate needs ~3.4us of sustained PE
            # activity to release 2.4GHz; zero-matmuls run while the
            # startup DMAs stream so the first real matmuls start warm.
            warm = const.tile([P, 640], bf16)
            nc.gpsimd.memset(warm[:], 0.0)
            for wi in range(10):
                wp = ps2.tile([P, 512], f32, tag="psy", name=f"warm{wi}")
                nc.tensor.matmul(wp[:], lhsT=warm[:, :128],
                                 rhs=warm[:, 128:640], start=True, stop=True)

            g_sb = const.tile([P, C // P], f32)
            if has_b1:
                b1_sb = const.tile([P, NB, 2, MP], f32)

            # per-block token tiles; block 0 is startup-critical and
            # rides the sync HWDGE queues (two triggers) alongside the
            # first W1 tile. DMA trigger instructions cost ~0.6us of
            # engine time each, so keep the startup trigger count small.
            xtb = [
                const.tile([P, KO1, TWs[bi]], bf16, name=f"xtb{bi}",
                           tag=f"xtb{bi}")
                for bi in range(NB)
            ]

            block_mm = {}     # (bi, mp) -> first matmul of that hidden tile

            def _stagger(dma_bi, anchor):
                if anchor is not None:
                    add_dep_helper(dma_bi.ins, anchor.ins, sync=True,
                                   reason="stagger bulk DMA behind compute")

            w1_tiles = {}

            def w1_load(bi, mp, split=False):
                t = w1p.tile([P, 2, KO1, P], bf16, tag="w1t",
                             name=f"w1_{bi}_{mp}")
                if split:
                    nc.sync.dma_start(t[:, 0], w1_d[bi, mp, :, 0])
                    nc.sync.dma_start(t[:, 1], w1_d[bi, mp, :, 1])
                else:
                    nc.sync.dma_start(t[:], w1_d[bi, mp])
                return t

            # startup-critical bytes as four sync triggers on parallel
            # HWDGE queues, ordered by first consumption: the first
            # matmul group (a 128-token sub-chunk of mp 0) needs only
            # the psa-half of w1t plus 128 token columns (~0.5MB).
            w1t00 = w1p.tile([P, 2, KO1, P], bf16, tag="w1t", name="w1_0_0")
            nc.sync.dma_start(w1t00[:, 0], w1_d[0, 0, :, 0])
            nc.sync.dma_start(xtb[0][:, :, 0:P], xt_d[:, :, 0:P])
            nc.sync.dma_start(w1t00[:, 1], w1_d[0, 0, :, 1])
            nc.sync.dma_start(xtb[0][:, :, P:TWs[0]], xt_d[:, :, P:TWs[0]])
            w1_tiles[(0, 0)] = w1t00

            hT_tiles = {}

            def get_hT(bi):
                if bi not in hT_tiles:
                    hT_tiles[bi] = ev.tile([P, MP, TWs[bi]], bf16, tag="hT",
                                           name=f"hT{bi}")
                return hT_tiles[bi]

            tagi = [0]

            def gemm1_mp(bi, mp):
                tw = TWs[bi]
                t0 = sum(TWs[:bi])
                hT = get_hT(bi)
                w1t = w1_tiles.pop((bi, mp), None)
                if w1t is None:
                    w1t = w1_load(bi, mp)
                if bi == 0 and mp == 0:
                    # startup: lead with a 128-token sub-chunk so the
                    # first matmuls issue as soon as ~0.5MB has landed.
                    # Its accumulators borrow the (idle) psy tag so the
                    # main 6-tag rotation sequence is unchanged.
                    chunks = [(0, P), (P, min(512 - P, tw - P))] + \
                        _chunks(tw)[1:]
                else:
                    chunks = _chunks(tw)
                for c0, cw in chunks:
                    if bi == 0 and mp == 0 and c0 == 0:
                        psa = ps2.tile([P, 512], f32, tag="psy",
                                       name="psa_boot")
                        psb = ps2.tile([P, 512], f32, tag="psy",
                                       name="psb_boot")
                    else:
                        psa = ps1.tile([P, 512], f32,
                                       tag=f"g1_{tagi[0] % 6}",
                                       name=f"psa_{bi}_{mp}_{c0}")
                        tagi[0] += 1
                        psb = ps1.tile([P, 512], f32,
                                       tag=f"g1_{tagi[0] % 6}",
                                       name=f"psb_{bi}_{mp}_{c0}")
                        tagi[0] += 1
                    for k in range(KO1):
                        mm = nc.tensor.matmul(
                            psa[:, :cw],
                            lhsT=w1t[:, 0, k, :],
                            rhs=xtb[bi][:, k, c0:c0 + cw],
                            start=(k == 0), stop=(k == KO1 - 1),
                        )
                        block_mm.setdefault((bi, mp), mm)
                    for k in range(KO1):
                        nc.tensor.matmul(
                            psb[:, :cw],
                            lhsT=w1t[:, 1, k, :],
                            rhs=xtb[bi][:, k, c0:c0 + cw],
                            start=(k == 0), stop=(k == KO1 - 1),
                        )
                    sg = ev.tile([P, 512], f32, tag="sg", bufs=3,
                                 name=f"sg_{bi}_{mp}_{c0}")
                    if has_b1:
                        nc.scalar.activation(sg[:, :cw], psa[:, :cw], SILU,
                                             bias=b1_sb[:, bi, 0, mp:mp + 1])
                        bs = ev.tile([P, 512], f32, tag="bs",
                                     name=f"bs_{bi}_{mp}_{c0}")
                        nc.vector.tensor_scalar_add(
                            bs[:, :cw], psb[:, :cw],
                            b1_sb[:, bi, 1, mp:mp + 1])
                        nc.vector.tensor_mul(hT[:, mp, c0:c0 + cw],
                                             sg[:, :cw], bs[:, :cw])
                    else:
                        nc.scalar.activation(sg[:, :cw], psa[:, :cw], SILU)
                        nc.vector.tensor_mul(hT[:, mp, c0:c0 + cw],
                                             sg[:, :cw], psb[:, :cw])

            w2_sb = {}
            for bi, tw in enumerate(TWs):
                t0 = sum(TWs[:bi])
                # ---- GEMM1 (mp 0 of bi>0 was emitted as the pipeline
                # filler before the previous block's GEMM2)
                for mp in range(1 if bi > 0 else 0, MP):
                    gemm1_mp(bi, mp)

                # ---- bulk loads during this block's GEMM1 window.
                # FIFO order on the gpsimd SWDGE path matters: this
                # block's W2 (needed at its GEMM2) goes FIRST, then
                # gates, then the other blocks' tokens (needed much
                # later).
                w2_sb[bi] = w2p.tile([P, KO2, D], bf16, tag="w2",
                                     name=f"w2_{bi}")
                kstep = max(1, KO2 // 4)
                for ci, k0 in enumerate(range(0, KO2, kstep)):
                    k1 = min(KO2, k0 + kstep)
                    dma = nc.gpsimd.dma_start(
                        w2_sb[bi][:, k0:k1, :], w2_d[bi, :, k0:k1, :])
                    anchor_mp = min(1 + 2 * ci, MP - 1)
                    _stagger(dma, block_mm.get((bi, anchor_mp)))
                if bi == 0:
                    dma = nc.gpsimd.dma_start(g_sb[:], g_d[:])
                    _stagger(dma, block_mm.get((0, 2)))
                    if has_b1:
                        dma = nc.gpsimd.dma_start(b1_sb[:], b1_d[:])
                        _stagger(dma, block_mm.get((0, 0)))
                    for nb in range(1, NB):
                        u0 = sum(TWs[:nb])
                        dma = nc.gpsimd.dma_start(
                            xtb[nb][:], xt_d[:, :, u0:u0 + TWs[nb]])
                        _stagger(dma, block_mm.get((0, min(8 + 3 * (nb - 1),
                                                           MP - 1))))

                # ---- prefetch next block's first W1 tiles on the sync
                # queue BEFORE this block's y DMAs join that FIFO
                if bi + 1 < NB:
                    for mp in range(W1_PREFETCH):
                        w1_tiles[(bi + 1, mp)] = w1_load(bi + 1, mp)
                    # pipeline filler: independent PE work while this
                    # block's last swiGLU drains into hT
                    gemm1_mp(bi + 1, 0)

                # ---- GEMM2 + gate scale; one y DMA per token m-tile ----
                for mt in range(tw // P):
                    ti = t0 // P + mt
                    rows = slice(t0 + mt * P, t0 + (mt + 1) * P)
                    ysb = ev.tile([P, D], f32, tag="ysb", bufs=3,
                                  name=f"ysb_{bi}_{mt}")
                    for n0, nw in n2chunks:
                        psy = ps2.tile([P, 512], f32, tag="psy",
                                       name=f"psy_{bi}_{mt}_{n0}")
                        for k in range(KO2):
                            nc.tensor.matmul(
                                psy[:, :nw],
                                lhsT=hT_tiles[bi][:, k, mt * P:(mt + 1) * P],
                                rhs=w2_sb[bi][:, k, n0:n0 + nw],
                                start=(k == 0), stop=(k == KO2 - 1),
                            )
                        nc.vector.tensor_scalar_mul(
                            ysb[:, n0:n0 + nw], psy[:, :nw], g_sb[:, ti:ti + 1]
                        )
                    nc.sync.dma_start(y_d[rows, :], ysb[:])
                del hT_tiles[bi]
    nc.finalize()
    return nc


def _route(x2, Wr):
    """Top-2 router, numpy fp32 (mirrors jax.lax.top_k + softmax)."""
    n = x2.shape[0]
    ar = np.arange(n)
    z = x2 @ Wr
    idx1 = z.argmax(axis=1)
    v1 = z[ar, idx1]
    z2 = z.copy()
    z2[ar, idx1] = -np.inf
    idx2 = z2.argmax(axis=1)
    v2 = z2[ar, idx2]
    m = np.maximum(v1, v2)
    e1 = np.exp(v1 - m)
    e2 = np.exp(v2 - m)
    s = e1 + e2
    return idx1, idx2, (e1 / s).astype(np.float32), (e2 / s).astype(np.float32)


def _pack_slots(tile_counts, ncores=NCORES):
    """Choose a per-core block pattern (same sizes on every core) and an
    expert label for every (core, block) slot such that each expert's
    128-token tiles are covered by whole slots. Returns (pattern, labels)
    with labels[core][block] = expert id."""
    E = len(tile_counts)
    ntc = max(1, math.ceil(sum(tile_counts) / ncores))
    for _ in range(64):
        r = ntc % 4
        if ntc >= 5 * r and ntc >= 4:
            n5, n4 = r, (ntc - 5 * r) // 4
            pattern = [4] * n4 + [5] * n5
        else:
            pattern = [ntc]
        avail = {sz: pattern.count(sz) * ncores for sz in set(pattern)}
        n5a = avail.get(5, 0)
        n4a = avail.get(4, 0)
        order = sorted(range(E), key=lambda e: -tile_counts[e])
        assign = {e: [] for e in range(E)}
        ok = True
        if len(pattern) == 1:
            for e in order:
                need = tile_counts[e]
                while need > 0:
                    if avail.get(pattern[0], 0) <= 0:
                        ok = False
                        break
                    avail[pattern[0]] -= 1
                    assign[e].append(pattern[0])
                    need -= pattern[0]
                if not ok:
                    break
        else:
            for e in order:
                need = tile_counts[e]
                if need == 0:
                    continue
                best = None
                for b in range(0, n5a + 1):
                    a = max(0, -(-(need - 5 * b) // 4))
                    if a > n4a:
                        continue
                    waste = 4 * a + 5 * b - need
                    if waste < 0:
                        continue
                    key = (waste, a + b)
                    if best is None or key < best[0]:
                        best = (key, a, b)
                if best is None:
                    ok = False
                    break
                _, a, b = best
                n4a -= a
                n5a -= b
                assign[e] = [4] * a + [5] * b
        if ok:
            by_size = {sz: [] for sz in set(pattern)}
            for e in range(E):
                for s in assign[e]:
                    by_size[s].append(e)
            for sz in set(pattern):
                total = pattern.count(sz) * ncores
                while len(by_size[sz]) < total:
                    by_size[sz].append(0)
            labels = []
            idx = {sz: 0 for sz in set(pattern)}
            for c in range(ncores):
                row = []
                for sz in pattern:
                    row.append(by_size[sz][idx[sz]])
                    idx[sz] += 1
                labels.append(row)
            return pattern, labels
        ntc += 1
    raise RuntimeError("slot packing failed")


def kernel(x, Wr, W1, b1, W2, b2):
    x = np.asarray(x, dtype=np.float32)
    Wr = np.asarray(Wr, dtype=np.float32)
    W1 = np.asarray(W1, dtype=np.float32)
    b1 = np.asarray(b1, dtype=np.float32)
    W2 = np.asarray(W2, dtype=np.float32)
    b2 = np.asarray(b2, dtype=np.float32)

    Bb, T, D = x.shape
    E, _, H2 = W1.shape
    H = H2 // 2
    N = Bb * T
    KO1 = D // P
    MP = H // P
    KO2 = H // P

    x2 = x.reshape(N, D)
    idx1, idx2, g1, g2 = _route(x2, Wr)

    tok = np.concatenate([np.arange(N), np.arange(N)])
    exp = np.concatenate([idx1, idx2])
    gat = np.concatenate([g1, g2])

    toks_e = [tok[exp == e] for e in range(E)]
    gats_e = [gat[exp == e] for e in range(E)]
    tiles = [math.ceil(len(t) / P) for t in toks_e]

    pattern, labels = _pack_slots(tiles)
    NB = len(pattern)
    TWs = [sz * P for sz in pattern]
    C = sum(TWs)

    slot_fill = {}
    cursor = [0] * E
    for c in range(NCORES):
        for b in range(NB):
            e = labels[c][b]
            cap = TWs[b]
            lo = cursor[e]
            hi = min(len(toks_e[e]), lo + cap)
            cursor[e] = hi
            slot_fill[(c, b)] = (toks_e[e][lo:hi], gats_e[e][lo:hi])
    for e in range(E):
        assert cursor[e] == len(toks_e[e]), "packing lost tokens"

    has_b1 = bool(np.any(b1))
    nc = build_moe_nc(D, H, TWs, has_b1=has_b1)

    x2b = x2.astype(np_bf16)
    w1T = [np.ascontiguousarray(
        W1[e].reshape(KO1, P, 2, MP, P).transpose(3, 1, 2, 0, 4)
    ).astype(np_bf16) for e in range(E)]
    w2T = [np.ascontiguousarray(
        W2[e].reshape(KO2, P, D).transpose(1, 0, 2)
    ).astype(np_bf16) for e in range(E)]

    in_maps = []
    for c in range(NCORES):
        xt = np.zeros((C, D), dtype=np_bf16)
        g = np.zeros(C, dtype=np.float32)
        t0 = 0
        for b in range(NB):
            tk, gt = slot_fill[(c, b)]
            xt[t0:t0 + len(tk)] = x2b[tk]
            g[t0:t0 + len(tk)] = gt
            t0 += TWs[b]
        xt_t = np.ascontiguousarray(
            xt.T.reshape(KO1, P, C).transpose(1, 0, 2))
        g_t = np.ascontiguousarray(g.reshape(C // P, P).T)
        w1s = np.stack([w1T[labels[c][b]] for b in range(NB)])
        w2s = np.stack([w2T[labels[c][b]] for b in range(NB)])
        im = {"xt": xt_t, "w1": w1s, "w2": w2s, "g": g_t}
        if has_b1:
            im["b1"] = np.ascontiguousarray(np.stack(
                [b1[labels[c][b]].reshape(2, MP, P) for b in range(NB)]
            ).transpose(3, 0, 1, 2))
        in_maps.append(im)

    res = run_bass_kernel_spmd(nc, in_maps, list(range(NCORES)))

    out = np.zeros((N, D), dtype=np.float32)
    for c in range(NCORES):
        y = res.results[c]["y"]
        t0 = 0
        for b in range(NB):
            tk, _ = slot_fill[(c, b)]
            if len(tk):
                np.add.at(out, tk, y[t0:t0 + len(tk)])
            t0 += TWs[b]

    if np.any(b2):
        comb = np.zeros((N, E), dtype=np.float32)
        comb[np.arange(N), idx1] += g1
        comb[np.arange(N), idx2] += g2
        out += comb @ b2
    return out.reshape(Bb, T, D)


# revision 33
# speedup vs baseline: 1.0059x; 1.0059x over previous
"""MoE layer (E=8 experts, top-2, swiGLU) on 8 TRN2 NeuronCores.

Strategy: balanced expert-block dispatch. The router runs on host; each
core is assigned a fixed pattern of token blocks (same block sizes on
every core -> one SPMD program), and each block is bound to ONE expert
whose weights are streamed per block from per-core DRAM data. Packing
experts into the 8x[4,4,4,5]-tile block grid balances the padded token
count to C = ceil(sum_e ceil(count_e/128) / 8)*128 per core instead of
max_e count (2176 vs 2304 for the balanced-random router here).

All matmul operands are bf16 (PE full rate, FWL weight loads hide under
the matmul stream, half the DMA bytes); accumulation stays fp32 in
PSUM, swiGLU runs fp32 on ACT (fused Silu) + DVE, hT is stored bf16,
outputs are fp32. Measured end-to-end error ~4e-3 of absmax (gate 2e-2).

Schedule notes (from trace iterations):
- Block 0's tokens + first W1 tile are the only startup-critical bytes:
  they ride the sync HWDGE queue alone; every bulk load (other blocks'
  tokens, W2, gates) rides SWDGE anchored behind real matmuls so it
  cannot steal HBM bandwidth from the critical path at t=0.
- PSUM: six 1-bank tags rotate through the per-chunk psa/psb
  accumulators (reuse distance 3 chunks ~ 10us), so the ACT/DVE swiGLU
  chain never back-pressures the PE via PSUM WAR.
- Before each block's GEMM2 (which must wait for that block's LAST
  swiGLU write into hT), the NEXT block's first GEMM1 hidden tile is
  emitted as a filler so the PE has independent work during the
  hT-finalize latency.
- W1 tiles of block bi+1 are prefetched on the sync queue BEFORE block
  bi's y DMAs join that FIFO (w1 pool bufs=6 makes the slots free early).
"""

import math

import numpy as np
import ml_dtypes

import concourse.bacc as bacc
import concourse.bass as bass  # noqa: F401
import concourse.mybir as mybir
import concourse.tile as tile
from concourse.bass_utils import run_bass_kernel_spmd
from concourse.tile import add_dep_helper

P = 128
NCORES = 8

f32 = mybir.dt.float32
bf16 = mybir.dt.bfloat16
np_bf16 = ml_dtypes.bfloat16
SILU = mybir.ActivationFunctionType.Silu


def _chunks(tw, step=512):
    out = []
    c0 = 0
    while c0 < tw:
        out.append((c0, min(step, tw - c0)))
        c0 += step
    return out


def build_moe_nc(D, H, TWs, has_b1=False):
    """One SPMD program: len(TWs) token blocks, sizes TWs (multiples of
    128), each block bound to its own W1/W2 slice of the per-core weight
    stream tensors."""
    KO1 = D // P       # GEMM1 contraction tiles (over D)
    MP = H // P        # hidden tiles (per swiGLU half)
    KO2 = H // P       # GEMM2 contraction tiles (over H)
    NB = len(TWs)
    C = sum(TWs)
    n2chunks = _chunks(D)

    nc = bacc.Bacc(None)
    xt_d = nc.declare_dram_parameter("xt", [P, KO1, C], bf16, isOutput=False)
    w1_d = nc.declare_dram_parameter(
        "w1", [NB, MP, P, 2, KO1, P], bf16, isOutput=False
    )
    w2_d = nc.declare_dram_parameter("w2", [NB, P, KO2, D], bf16, isOutput=False)
    g_d = nc.declare_dram_parameter("g", [P, C // P], f32, isOutput=False)
    if has_b1:
        b1_d = nc.declare_dram_parameter("b1", [P, NB, 2, MP], f32, isOutput=False)
    y_d = nc.declare_dram_parameter("y", [C, D], f32, isOutput=True)

    W1_PREFETCH = 3

    with tile.TileContext(nc) as tc:
        with (
            tc.tile_pool(name="const", bufs=1) as const,
            tc.tile_pool(name="w1p", bufs=6) as w1p,
            tc.tile_pool(name="w2p", bufs=2) as w2p,
            tc.tile_pool(name="ev", bufs=2) as ev,
            tc.tile_pool(name="ps1", bufs=1, space="PSUM") as ps1,
            tc.tile_pool(name="ps2", bufs=2, space="PSUM") as ps2,
        ):
            # PE warmup: the HAM clock gate needs ~3.4us of sustained PE
            # activity to release 2.4GHz; zero-matmuls run while the
            # startup DMAs stream so the first real matmuls start warm.
            warm = const.tile([P, 640], bf16)
            nc.gpsimd.memset(warm[:], 0.0)
            for wi in range(10):
                wp = ps2.tile([P, 512], f32, tag="psy", name=f"warm{wi}")
                nc.tensor.matmul(wp[:], lhsT=warm[:, :128],
                                 rhs=warm[:, 128:640], start=True, stop=True)

            g_sb = const.tile([P, C // P], f32)
            if has_b1:
                b1_sb = const.tile([P, NB, 2, MP], f32)

            # per-block token tiles; block 0 is startup-critical: two
            # SWDGE triggers (gpsimd engine) run in parallel with the
            # sync-queue W1 tile load. DMA trigger instructions cost
            # ~0.6us of engine time each, so keep the count small and
            # split across engines.
            xtb = [
                const.tile([P, KO1, TWs[bi]], bf16, name=f"xtb{bi}",
                           tag=f"xtb{bi}")
                for bi in range(NB)
            ]

            block_mm = {}     # (bi, mp) -> first matmul of that hidden tile

            def _stagger(dma_bi, anchor):
                if anchor is not None:
                    add_dep_helper(dma_bi.ins, anchor.ins, sync=True,
                                   reason="stagger bulk DMA behind compute")

            w1_tiles = {}

            def w1_load(bi, mp, split=False):
                t = w1p.tile([P, 2, KO1, P], bf16, tag="w1t",
                             name=f"w1_{bi}_{mp}")
                if split:
                    nc.sync.dma_start(t[:, 0], w1_d[bi, mp, :, 0])
                    nc.sync.dma_start(t[:, 1], w1_d[bi, mp, :, 1])
                else:
                    nc.sync.dma_start(t[:], w1_d[bi, mp])
                return t

            # startup-critical bytes: first W1 tile + block 0 tokens as
            # three sync triggers -> three HWDGE queues transfer in
            # parallel; the first matmul group starts after w1t + the
            # first k-half of the tokens (~1MB).
            w1_tiles[(0, 0)] = w1_load(0, 0)
            kh = KO1 // 2
            nc.sync.dma_start(xtb[0][:, :kh, :], xt_d[:, :kh, 0:TWs[0]])
            nc.sync.dma_start(xtb[0][:, kh:, :], xt_d[:, kh:, 0:TWs[0]])

            hT_tiles = {}

            def get_hT(bi):
                if bi not in hT_tiles:
                    hT_tiles[bi] = ev.tile([P, MP, TWs[bi]], bf16, tag="hT",
                                           name=f"hT{bi}")
                return hT_tiles[bi]

            tagi = [0]

            def gemm1_mp(bi, mp):
                tw = TWs[bi]
                t0 = sum(TWs[:bi])
                hT = get_hT(bi)
                w1t = w1_tiles.pop((bi, mp), None)
                if w1t is None:
                    w1t = w1_load(bi, mp)
                for c0, cw in _chunks(tw):
                    psa = ps1.tile([P, 512], f32, tag=f"g1_{tagi[0] % 6}",
                                   name=f"psa_{bi}_{mp}_{c0}")
                    tagi[0] += 1
                    psb = ps1.tile([P, 512], f32, tag=f"g1_{tagi[0] % 6}",
                                   name=f"psb_{bi}_{mp}_{c0}")
                    tagi[0] += 1
                    for k in range(KO1):
                        mm = nc.tensor.matmul(
                            psa[:, :cw],
                            lhsT=w1t[:, 0, k, :],
                            rhs=xtb[bi][:, k, c0:c0 + cw],
                            start=(k == 0), stop=(k == KO1 - 1),
                        )
                        block_mm.setdefault((bi, mp), mm)
                    for k in range(KO1):
                        nc.tensor.matmul(
                            psb[:, :cw],
                            lhsT=w1t[:, 1, k, :],
                            rhs=xtb[bi][:, k, c0:c0 + cw],
                            start=(k == 0), stop=(k == KO1 - 1),
                        )
                    sg = ev.tile([P, 512], f32, tag="sg", bufs=3,
                                 name=f"sg_{bi}_{mp}_{c0}")
                    if has_b1:
                        nc.scalar.activation(sg[:, :cw], psa[:, :cw], SILU,
                                             bias=b1_sb[:, bi, 0, mp:mp + 1])
                        bs = ev.tile([P, 512], f32, tag="bs",
                                     name=f"bs_{bi}_{mp}_{c0}")
                        nc.vector.tensor_scalar_add(
                            bs[:, :cw], psb[:, :cw],
                            b1_sb[:, bi, 1, mp:mp + 1])
                        nc.vector.tensor_mul(hT[:, mp, c0:c0 + cw],
                                             sg[:, :cw], bs[:, :cw])
                    else:
                        nc.scalar.activation(sg[:, :cw], psa[:, :cw], SILU)
                        nc.vector.tensor_mul(hT[:, mp, c0:c0 + cw],
                                             sg[:, :cw], psb[:, :cw])

            w2_sb = {}
            for bi, tw in enumerate(TWs):
                t0 = sum(TWs[:bi])
                # ---- GEMM1 (mp 0 of bi>0 was emitted as the pipeline
                # filler before the previous block's GEMM2)
                for mp in range(1 if bi > 0 else 0, MP):
                    gemm1_mp(bi, mp)

                # ---- bulk loads during this block's GEMM1 window.
                # FIFO order on the gpsimd SWDGE path matters: this
                # block's W2 (needed at its GEMM2) goes FIRST, then
                # gates, then the other blocks' tokens (needed much
                # later).
                w2_sb[bi] = w2p.tile([P, KO2, D], bf16, tag="w2",
                                     name=f"w2_{bi}")
                kstep = max(1, KO2 // 4)
                for ci, k0 in enumerate(range(0, KO2, kstep)):
                    k1 = min(KO2, k0 + kstep)
                    dma = nc.gpsimd.dma_start(
                        w2_sb[bi][:, k0:k1, :], w2_d[bi, :, k0:k1, :])
                    anchor_mp = min(1 + 2 * ci, MP - 1)
                    _stagger(dma, block_mm.get((bi, anchor_mp)))
                if bi == 0:
                    dma = nc.gpsimd.dma_start(g_sb[:], g_d[:])
                    _stagger(dma, block_mm.get((0, 2)))
                    if has_b1:
                        dma = nc.gpsimd.dma_start(b1_sb[:], b1_d[:])
                        _stagger(dma, block_mm.get((0, 0)))
                    for nb in range(1, NB):
                        u0 = sum(TWs[:nb])
                        dma = nc.gpsimd.dma_start(
                            xtb[nb][:], xt_d[:, :, u0:u0 + TWs[nb]])
                        _stagger(dma, block_mm.get((0, min(8 + 3 * (nb - 1),
                                                           MP - 1))))

                # ---- prefetch next block's first W1 tiles on the sync
                # queue BEFORE this block's y DMAs join that FIFO
                if bi + 1 < NB:
                    for mp in range(W1_PREFETCH):
                        w1_tiles[(bi + 1, mp)] = w1_load(bi + 1, mp)
                    # pipeline filler: independent PE work while this
                    # block's last swiGLU drains into hT
                    gemm1_mp(bi + 1, 0)

                # ---- GEMM2 + gate scale; one y DMA per token m-tile ----
                for mt in range(tw // P):
                    ti = t0 // P + mt
                    rows = slice(t0 + mt * P, t0 + (mt + 1) * P)
                    ysb = ev.tile([P, D], f32, tag="ysb", bufs=3,
                                  name=f"ysb_{bi}_{mt}")
                    for n0, nw in n2chunks:
                        psy = ps2.tile([P, 512], f32, tag="psy",
                                       name=f"psy_{bi}_{mt}_{n0}")
                        for k in range(KO2):
                            nc.tensor.matmul(
                                psy[:, :nw],
                                lhsT=hT_tiles[bi][:, k, mt * P:(mt + 1) * P],
                                rhs=w2_sb[bi][:, k, n0:n0 + nw],
                                start=(k == 0), stop=(k == KO2 - 1),
                            )
                        nc.vector.tensor_scalar_mul(
                            ysb[:, n0:n0 + nw], psy[:, :nw], g_sb[:, ti:ti + 1]
                        )
                    nc.sync.dma_start(y_d[rows, :], ysb[:])
                del hT_tiles[bi]
    nc.finalize()
    return nc


def _route(x2, Wr):
    """Top-2 router, numpy fp32 (mirrors jax.lax.top_k + softmax)."""
    n = x2.shape[0]
    ar = np.arange(n)
    z = x2 @ Wr
    idx1 = z.argmax(axis=1)
    v1 = z[ar, idx1]
    z2 = z.copy()
    z2[ar, idx1] = -np.inf
    idx2 = z2.argmax(axis=1)
    v2 = z2[ar, idx2]
    m = np.maximum(v1, v2)
    e1 = np.exp(v1 - m)
    e2 = np.exp(v2 - m)
    s = e1 + e2
    return idx1, idx2, (e1 / s).astype(np.float32), (e2 / s).astype(np.float32)


def _pack_slots(tile_counts, ncores=NCORES):
    """Choose a per-core block pattern (same sizes on every core) and an
    expert label for every (core, block) slot such that each expert's
    128-token tiles are covered by whole slots. Returns (pattern, labels)
    with labels[core][block] = expert id."""
    E = len(tile_counts)
    ntc = max(1, math.ceil(sum(tile_counts) / ncores))
    for _ in range(64):
        r = ntc % 4
        if ntc >= 5 * r and ntc >= 4:
            n5, n4 = r, (ntc - 5 * r) // 4
            pattern = [4] * n4 + [5] * n5
        else:
            pattern = [ntc]
        avail = {sz: pattern.count(sz) * ncores for sz in set(pattern)}
        n5a = avail.get(5, 0)
        n4a = avail.get(4, 0)
        order = sorted(range(E), key=lambda e: -tile_counts[e])
        assign = {e: [] for e in range(E)}
        ok = True
        if len(pattern) == 1:
            for e in order:
                need = tile_counts[e]
                while need > 0:
                    if avail.get(pattern[0], 0) <= 0:
                        ok = False
                        break
                    avail[pattern[0]] -= 1
                    assign[e].append(pattern[0])
                    need -= pattern[0]
                if not ok:
                    break
        else:
            for e in order:
                need = tile_counts[e]
                if need == 0:
                    continue
                best = None
                for b in range(0, n5a + 1):
                    a = max(0, -(-(need - 5 * b) // 4))
                    if a > n4a:
                        continue
                    waste = 4 * a + 5 * b - need
                    if waste < 0:
                        continue
                    key = (waste, a + b)
                    if best is None or key < best[0]:
                        best = (key, a, b)
                if best is None:
                    ok = False
                    break
                _, a, b = best
                n4a -= a
                n5a -= b
                assign[e] = [4] * a + [5] * b
        if ok:
            by_size = {sz: [] for sz in set(pattern)}
            for e in range(E):
                for s in assign[e]:
                    by_size[s].append(e)
            for sz in set(pattern):
                total = pattern.count(sz) * ncores
                while len(by_size[sz]) < total:
                    by_size[sz].append(0)
            labels = []
            idx = {sz: 0 for sz in set(pattern)}
            for c in range(ncores):
                row = []
                for sz in pattern:
                    row.append(by_size[sz][idx[sz]])
                    idx[sz] += 1
                labels.append(row)
            return pattern, labels
        ntc += 1
    raise RuntimeError("slot packing failed")


def kernel(x, Wr, W1, b1, W2, b2):
    x = np.asarray(x, dtype=np.float32)
    Wr = np.asarray(Wr, dtype=np.float32)
    W1 = np.asarray(W1, dtype=np.float32)
    b1 = np.asarray(b1, dtype=np.float32)
    W2 = np.asarray(W2, dtype=np.float32)
    b2 = np.asarray(b2, dtype=np.float32)

    Bb, T, D = x.shape
    E, _, H2 = W1.shape
    H = H2 // 2
    N = Bb * T
    KO1 = D // P
    MP = H // P
    KO2 = H // P

    x2 = x.reshape(N, D)
    idx1, idx2, g1, g2 = _route(x2, Wr)

    tok = np.concatenate([np.arange(N), np.arange(N)])
    exp = np.concatenate([idx1, idx2])
    gat = np.concatenate([g1, g2])

    toks_e = [tok[exp == e] for e in range(E)]
    gats_e = [gat[exp == e] for e in range(E)]
    tiles = [math.ceil(len(t) / P) for t in toks_e]

    pattern, labels = _pack_slots(tiles)
    NB = len(pattern)
    TWs = [sz * P for sz in pattern]
    C = sum(TWs)

    slot_fill = {}
    cursor = [0] * E
    for c in range(NCORES):
        for b in range(NB):
            e = labels[c][b]
            cap = TWs[b]
            lo = cursor[e]
            hi = min(len(toks_e[e]), lo + cap)
            cursor[e] = hi
            slot_fill[(c, b)] = (toks_e[e][lo:hi], gats_e[e][lo:hi])
    for e in range(E):
        assert cursor[e] == len(toks_e[e]), "packing lost tokens"

    has_b1 = bool(np.any(b1))
    nc = build_moe_nc(D, H, TWs, has_b1=has_b1)

    x2b = x2.astype(np_bf16)
    w1T = [np.ascontiguousarray(
        W1[e].reshape(KO1, P, 2, MP, P).transpose(3, 1, 2, 0, 4)
    ).astype(np_bf16) for e in range(E)]
    w2T = [np.ascontiguousarray(
        W2[e].reshape(KO2, P, D).transpose(1, 0, 2)
    ).astype(np_bf16) for e in range(E)]

    in_maps = []
    for c in range(NCORES):
        xt = np.zeros((C, D), dtype=np_bf16)
        g = np.zeros(C, dtype=np.float32)
        t0 = 0
        for b in range(NB):
            tk, gt = slot_fill[(c, b)]
            xt[t0:t0 + len(tk)] = x2b[tk]
            g[t0:t0 + len(tk)] = gt
            t0 += TWs[b]
        xt_t = np.ascontiguousarray(
            xt.T.reshape(KO1, P, C).transpose(1, 0, 2))
        g_t = np.ascontiguousarray(g.reshape(C // P, P).T)
        w1s = np.stack([w1T[labels[c][b]] for b in range(NB)])
        w2s = np.stack([w2T[labels[c][b]] for b in range(NB)])
        im = {"xt": xt_t, "w1": w1s, "w2": w2s, "g": g_t}
        if has_b1:
            im["b1"] = np.ascontiguousarray(np.stack(
                [b1[labels[c][b]].reshape(2, MP, P) for b in range(NB)]
            ).transpose(3, 0, 1, 2))
        in_maps.append(im)

    res = run_bass_kernel_spmd(nc, in_maps, list(range(NCORES)))

    out = np.zeros((N, D), dtype=np.float32)
    for c in range(NCORES):
        y = res.results[c]["y"]
        t0 = 0
        for b in range(NB):
            tk, _ = slot_fill[(c, b)]
            if len(tk):
                np.add.at(out, tk, y[t0:t0 + len(tk)])
            t0 += TWs[b]

    if np.any(b2):
        comb = np.zeros((N, E), dtype=np.float32)
        comb[np.arange(N), idx1] += g1
        comb[np.arange(N), idx2] += g2
        out += comb @ b2
    return out.reshape(Bb, T, D)


# revision 34
# speedup vs baseline: 1.0132x; 1.0073x over previous
"""MoE layer (E=8 experts, top-2, swiGLU) on 8 TRN2 NeuronCores.

Strategy: balanced expert-block dispatch. The router runs on host; each
core is assigned a fixed pattern of token blocks (same block sizes on
every core -> one SPMD program), and each block is bound to ONE expert
whose weights are streamed per block from per-core DRAM data. Packing
experts into the 8x[4,4,4,5]-tile block grid balances the padded token
count to C = ceil(sum_e ceil(count_e/128) / 8)*128 per core instead of
max_e count (2176 vs 2304 for the balanced-random router here).

All matmul operands are bf16 (PE full rate, FWL weight loads hide under
the matmul stream, half the DMA bytes); accumulation stays fp32 in
PSUM, swiGLU runs fp32 on ACT (fused Silu) + DVE, hT is stored bf16,
outputs are fp32. Measured end-to-end error ~4e-3 of absmax (gate 2e-2).

Schedule notes (from trace iterations):
- Block 0's tokens + first W1 tile are the only startup-critical bytes:
  they ride the sync HWDGE queue alone; every bulk load (other blocks'
  tokens, W2, gates) rides SWDGE anchored behind real matmuls so it
  cannot steal HBM bandwidth from the critical path at t=0.
- PSUM: six 1-bank tags rotate through the per-chunk psa/psb
  accumulators (reuse distance 3 chunks ~ 10us), so the ACT/DVE swiGLU
  chain never back-pressures the PE via PSUM WAR.
- Before each block's GEMM2 (which must wait for that block's LAST
  swiGLU write into hT), the NEXT block's first GEMM1 hidden tile is
  emitted as a filler so the PE has independent work during the
  hT-finalize latency.
- W1 tiles of block bi+1 are prefetched on the sync queue BEFORE block
  bi's y DMAs join that FIFO (w1 pool bufs=6 makes the slots free early).
"""

import math

import numpy as np
import ml_dtypes

import concourse.bacc as bacc
import concourse.bass as bass  # noqa: F401
import concourse.mybir as mybir
import concourse.tile as tile
from concourse.bass_utils import run_bass_kernel_spmd
from concourse.tile import add_dep_helper

P = 128
NCORES = 8

f32 = mybir.dt.float32
bf16 = mybir.dt.bfloat16
np_bf16 = ml_dtypes.bfloat16
SILU = mybir.ActivationFunctionType.Silu


def _chunks(tw, step=512):
    out = []
    c0 = 0
    while c0 < tw:
        out.append((c0, min(step, tw - c0)))
        c0 += step
    return out


def build_moe_nc(D, H, TWs, has_b1=False):
    """One SPMD program: len(TWs) token blocks, sizes TWs (multiples of
    128), each block bound to its own W1/W2 slice of the per-core weight
    stream tensors."""
    KO1 = D // P       # GEMM1 contraction tiles (over D)
    MP = H // P        # hidden tiles (per swiGLU half)
    KO2 = H // P       # GEMM2 contraction tiles (over H)
    NB = len(TWs)
    C = sum(TWs)
    n2chunks = _chunks(D)

    nc = bacc.Bacc(None)
    xt_d = nc.declare_dram_parameter("xt", [P, KO1, C], bf16, isOutput=False)
    w1_d = nc.declare_dram_parameter(
        "w1", [NB, MP, P, 2, KO1, P], bf16, isOutput=False
    )
    w2_d = nc.declare_dram_parameter("w2", [NB, P, KO2, D], bf16, isOutput=False)
    g_d = nc.declare_dram_parameter("g", [P, C // P], f32, isOutput=False)
    if has_b1:
        b1_d = nc.declare_dram_parameter("b1", [P, NB, 2, MP], f32, isOutput=False)
    y_d = nc.declare_dram_parameter("y", [C, D], f32, isOutput=True)

    W1_PREFETCH = 3

    with tile.TileContext(nc) as tc:
        with (
            tc.tile_pool(name="const", bufs=1) as const,
            tc.tile_pool(name="w1p", bufs=6) as w1p,
            tc.tile_pool(name="w2p", bufs=2) as w2p,
            tc.tile_pool(name="ev", bufs=2) as ev,
            tc.tile_pool(name="ps1", bufs=1, space="PSUM") as ps1,
            tc.tile_pool(name="ps2", bufs=2, space="PSUM") as ps2,
        ):
            # PE warmup: the HAM clock gate needs ~3.4us of sustained PE
            # activity to release 2.4GHz; zero-matmuls run while the
            # startup DMAs stream so the first real matmuls start warm.
            warm = const.tile([P, 640], bf16)
            nc.gpsimd.memset(warm[:], 0.0)
            for wi in range(13):
                wp = ps2.tile([P, 512], f32, tag="psy", name=f"warm{wi}")
                nc.tensor.matmul(wp[:], lhsT=warm[:, :128],
                                 rhs=warm[:, 128:640], start=True, stop=True)

            g_sb = const.tile([P, C // P], f32)
            if has_b1:
                b1_sb = const.tile([P, NB, 2, MP], f32)

            # per-block token tiles; block 0 is startup-critical: two
            # SWDGE triggers (gpsimd engine) run in parallel with the
            # sync-queue W1 tile load. DMA trigger instructions cost
            # ~0.6us of engine time each, so keep the count small and
            # split across engines.
            xtb = [
                const.tile([P, KO1, TWs[bi]], bf16, name=f"xtb{bi}",
                           tag=f"xtb{bi}")
                for bi in range(NB)
            ]

            block_mm = {}     # (bi, mp) -> first matmul of that hidden tile

            def _stagger(dma_bi, anchor):
                if anchor is not None:
                    add_dep_helper(dma_bi.ins, anchor.ins, sync=True,
                                   reason="stagger bulk DMA behind compute")

            w1_tiles = {}

            def w1_load(bi, mp, split=False):
                t = w1p.tile([P, 2, KO1, P], bf16, tag="w1t",
                             name=f"w1_{bi}_{mp}")
                if split:
                    nc.sync.dma_start(t[:, 0], w1_d[bi, mp, :, 0])
                    nc.sync.dma_start(t[:, 1], w1_d[bi, mp, :, 1])
                else:
                    nc.sync.dma_start(t[:], w1_d[bi, mp])
                return t

            # startup-critical bytes: first W1 tile + block 0 tokens as
            # three sync triggers -> three HWDGE queues transfer in
            # parallel; the first matmul group starts after w1t + the
            # first k-half of the tokens (~1MB).
            w1_tiles[(0, 0)] = w1_load(0, 0)
            kh = KO1 // 2
            nc.sync.dma_start(xtb[0][:, :kh, :], xt_d[:, :kh, 0:TWs[0]])
            nc.sync.dma_start(xtb[0][:, kh:, :], xt_d[:, kh:, 0:TWs[0]])

            hT_tiles = {}

            def get_hT(bi):
                if bi not in hT_tiles:
                    hT_tiles[bi] = ev.tile([P, MP, TWs[bi]], bf16, tag="hT",
                                           name=f"hT{bi}")
                return hT_tiles[bi]

            tagi = [0]

            def gemm1_mp(bi, mp):
                tw = TWs[bi]
                t0 = sum(TWs[:bi])
                hT = get_hT(bi)
                w1t = w1_tiles.pop((bi, mp), None)
                if w1t is None:
                    w1t = w1_load(bi, mp)
                for c0, cw in _chunks(tw):
                    psa = ps1.tile([P, 512], f32, tag=f"g1_{tagi[0] % 6}",
                                   name=f"psa_{bi}_{mp}_{c0}")
                    tagi[0] += 1
                    psb = ps1.tile([P, 512], f32, tag=f"g1_{tagi[0] % 6}",
                                   name=f"psb_{bi}_{mp}_{c0}")
                    tagi[0] += 1
                    for k in range(KO1):
                        mm = nc.tensor.matmul(
                            psa[:, :cw],
                            lhsT=w1t[:, 0, k, :],
                            rhs=xtb[bi][:, k, c0:c0 + cw],
                            start=(k == 0), stop=(k == KO1 - 1),
                        )
                        block_mm.setdefault((bi, mp), mm)
                    for k in range(KO1):
                        nc.tensor.matmul(
                            psb[:, :cw],
                            lhsT=w1t[:, 1, k, :],
                            rhs=xtb[bi][:, k, c0:c0 + cw],
                            start=(k == 0), stop=(k == KO1 - 1),
                        )
                    sg = ev.tile([P, 512], f32, tag="sg", bufs=3,
                                 name=f"sg_{bi}_{mp}_{c0}")
                    if has_b1:
                        nc.scalar.activation(sg[:, :cw], psa[:, :cw], SILU,
                                             bias=b1_sb[:, bi, 0, mp:mp + 1])
                        bs = ev.tile([P, 512], f32, tag="bs",
                                     name=f"bs_{bi}_{mp}_{c0}")
                        nc.vector.tensor_scalar_add(
                            bs[:, :cw], psb[:, :cw],
                            b1_sb[:, bi, 1, mp:mp + 1])
                        nc.vector.tensor_mul(hT[:, mp, c0:c0 + cw],
                                             sg[:, :cw], bs[:, :cw])
                    else:
                        nc.scalar.activation(sg[:, :cw], psa[:, :cw], SILU)
                        nc.vector.tensor_mul(hT[:, mp, c0:c0 + cw],
                                             sg[:, :cw], psb[:, :cw])

            w2_sb = {}
            for bi, tw in enumerate(TWs):
                t0 = sum(TWs[:bi])
                # ---- GEMM1 (mp 0 of bi>0 was emitted as the pipeline
                # filler before the previous block's GEMM2)
                for mp in range(1 if bi > 0 else 0, MP):
                    gemm1_mp(bi, mp)

                # ---- bulk loads during this block's GEMM1 window.
                # FIFO order on the gpsimd SWDGE path matters: this
                # block's W2 (needed at its GEMM2) goes FIRST, then
                # gates, then the other blocks' tokens (needed much
                # later).
                w2_sb[bi] = w2p.tile([P, KO2, D], bf16, tag="w2",
                                     name=f"w2_{bi}")
                kstep = max(1, KO2 // 4)
                for ci, k0 in enumerate(range(0, KO2, kstep)):
                    k1 = min(KO2, k0 + kstep)
                    dma = nc.gpsimd.dma_start(
                        w2_sb[bi][:, k0:k1, :], w2_d[bi, :, k0:k1, :])
                    anchor_mp = min(1 + 2 * ci, MP - 1)
                    _stagger(dma, block_mm.get((bi, anchor_mp)))
                if bi == 0:
                    dma = nc.gpsimd.dma_start(g_sb[:], g_d[:])
                    _stagger(dma, block_mm.get((0, 2)))
                    if has_b1:
                        dma = nc.gpsimd.dma_start(b1_sb[:], b1_d[:])
                        _stagger(dma, block_mm.get((0, 0)))
                    for nb in range(1, NB):
                        u0 = sum(TWs[:nb])
                        dma = nc.gpsimd.dma_start(
                            xtb[nb][:], xt_d[:, :, u0:u0 + TWs[nb]])
                        _stagger(dma, block_mm.get((0, min(8 + 3 * (nb - 1),
                                                           MP - 1))))

                # ---- prefetch next block's first W1 tiles on the sync
                # queue BEFORE this block's y DMAs join that FIFO
                if bi + 1 < NB:
                    for mp in range(W1_PREFETCH):
                        w1_tiles[(bi + 1, mp)] = w1_load(bi + 1, mp)
                    # pipeline filler: independent PE work while this
                    # block's last swiGLU drains into hT
                    gemm1_mp(bi + 1, 0)

                # ---- GEMM2 + gate scale; one y DMA per token m-tile ----
                for mt in range(tw // P):
                    ti = t0 // P + mt
                    rows = slice(t0 + mt * P, t0 + (mt + 1) * P)
                    ysb = ev.tile([P, D], f32, tag="ysb", bufs=3,
                                  name=f"ysb_{bi}_{mt}")
                    for n0, nw in n2chunks:
                        psy = ps2.tile([P, 512], f32, tag="psy",
                                       name=f"psy_{bi}_{mt}_{n0}")
                        for k in range(KO2):
                            nc.tensor.matmul(
                                psy[:, :nw],
                                lhsT=hT_tiles[bi][:, k, mt * P:(mt + 1) * P],
                                rhs=w2_sb[bi][:, k, n0:n0 + nw],
                                start=(k == 0), stop=(k == KO2 - 1),
                            )
                        nc.vector.tensor_scalar_mul(
                            ysb[:, n0:n0 + nw], psy[:, :nw], g_sb[:, ti:ti + 1]
                        )
                    nc.sync.dma_start(y_d[rows, :], ysb[:])
                del hT_tiles[bi]
    nc.finalize()
    return nc


def _route(x2, Wr):
    """Top-2 router, numpy fp32 (mirrors jax.lax.top_k + softmax)."""
    n = x2.shape[0]
    ar = np.arange(n)
    z = x2 @ Wr
    idx1 = z.argmax(axis=1)
    v1 = z[ar, idx1]
    z2 = z.copy()
    z2[ar, idx1] = -np.inf
    idx2 = z2.argmax(axis=1)
    v2 = z2[ar, idx2]
    m = np.maximum(v1, v2)
    e1 = np.exp(v1 - m)
    e2 = np.exp(v2 - m)
    s = e1 + e2
    return idx1, idx2, (e1 / s).astype(np.float32), (e2 / s).astype(np.float32)


def _pack_slots(tile_counts, ncores=NCORES):
    """Choose a per-core block pattern (same sizes on every core) and an
    expert label for every (core, block) slot such that each expert's
    128-token tiles are covered by whole slots. Returns (pattern, labels)
    with labels[core][block] = expert id."""
    E = len(tile_counts)
    ntc = max(1, math.ceil(sum(tile_counts) / ncores))
    for _ in range(64):
        r = ntc % 4
        if ntc >= 5 * r and ntc >= 4:
            n5, n4 = r, (ntc - 5 * r) // 4
            pattern = [4] * n4 + [5] * n5
        else:
            pattern = [ntc]
        avail = {sz: pattern.count(sz) * ncores for sz in set(pattern)}
        n5a = avail.get(5, 0)
        n4a = avail.get(4, 0)
        order = sorted(range(E), key=lambda e: -tile_counts[e])
        assign = {e: [] for e in range(E)}
        ok = True
        if len(pattern) == 1:
            for e in order:
                need = tile_counts[e]
                while need > 0:
                    if avail.get(pattern[0], 0) <= 0:
                        ok = False
                        break
                    avail[pattern[0]] -= 1
                    assign[e].append(pattern[0])
                    need -= pattern[0]
                if not ok:
                    break
        else:
            for e in order:
                need = tile_counts[e]
                if need == 0:
                    continue
                best = None
                for b in range(0, n5a + 1):
                    a = max(0, -(-(need - 5 * b) // 4))
                    if a > n4a:
                        continue
                    waste = 4 * a + 5 * b - need
                    if waste < 0:
                        continue
                    key = (waste, a + b)
                    if best is None or key < best[0]:
                        best = (key, a, b)
                if best is None:
                    ok = False
                    break
                _, a, b = best
                n4a -= a
                n5a -= b
                assign[e] = [4] * a + [5] * b
        if ok:
            by_size = {sz: [] for sz in set(pattern)}
            for e in range(E):
                for s in assign[e]:
                    by_size[s].append(e)
            for sz in set(pattern):
                total = pattern.count(sz) * ncores
                while len(by_size[sz]) < total:
                    by_size[sz].append(0)
            labels = []
            idx = {sz: 0 for sz in set(pattern)}
            for c in range(ncores):
                row = []
                for sz in pattern:
                    row.append(by_size[sz][idx[sz]])
                    idx[sz] += 1
                labels.append(row)
            return pattern, labels
        ntc += 1
    raise RuntimeError("slot packing failed")


def kernel(x, Wr, W1, b1, W2, b2):
    x = np.asarray(x, dtype=np.float32)
    Wr = np.asarray(Wr, dtype=np.float32)
    W1 = np.asarray(W1, dtype=np.float32)
    b1 = np.asarray(b1, dtype=np.float32)
    W2 = np.asarray(W2, dtype=np.float32)
    b2 = np.asarray(b2, dtype=np.float32)

    Bb, T, D = x.shape
    E, _, H2 = W1.shape
    H = H2 // 2
    N = Bb * T
    KO1 = D // P
    MP = H // P
    KO2 = H // P

    x2 = x.reshape(N, D)
    idx1, idx2, g1, g2 = _route(x2, Wr)

    tok = np.concatenate([np.arange(N), np.arange(N)])
    exp = np.concatenate([idx1, idx2])
    gat = np.concatenate([g1, g2])

    toks_e = [tok[exp == e] for e in range(E)]
    gats_e = [gat[exp == e] for e in range(E)]
    tiles = [math.ceil(len(t) / P) for t in toks_e]

    pattern, labels = _pack_slots(tiles)
    NB = len(pattern)
    TWs = [sz * P for sz in pattern]
    C = sum(TWs)

    slot_fill = {}
    cursor = [0] * E
    for c in range(NCORES):
        for b in range(NB):
            e = labels[c][b]
            cap = TWs[b]
            lo = cursor[e]
            hi = min(len(toks_e[e]), lo + cap)
            cursor[e] = hi
            slot_fill[(c, b)] = (toks_e[e][lo:hi], gats_e[e][lo:hi])
    for e in range(E):
        assert cursor[e] == len(toks_e[e]), "packing lost tokens"

    has_b1 = bool(np.any(b1))
    nc = build_moe_nc(D, H, TWs, has_b1=has_b1)

    x2b = x2.astype(np_bf16)
    w1T = [np.ascontiguousarray(
        W1[e].reshape(KO1, P, 2, MP, P).transpose(3, 1, 2, 0, 4)
    ).astype(np_bf16) for e in range(E)]
    w2T = [np.ascontiguousarray(
        W2[e].reshape(KO2, P, D).transpose(1, 0, 2)
    ).astype(np_bf16) for e in range(E)]

    in_maps = []
    for c in range(NCORES):
        xt = np.zeros((C, D), dtype=np_bf16)
        g = np.zeros(C, dtype=np.float32)
        t0 = 0
        for b in range(NB):
            tk, gt = slot_fill[(c, b)]
            xt[t0:t0 + len(tk)] = x2b[tk]
            g[t0:t0 + len(tk)] = gt
            t0 += TWs[b]
        xt_t = np.ascontiguousarray(
            xt.T.reshape(KO1, P, C).transpose(1, 0, 2))
        g_t = np.ascontiguousarray(g.reshape(C // P, P).T)
        w1s = np.stack([w1T[labels[c][b]] for b in range(NB)])
        w2s = np.stack([w2T[labels[c][b]] for b in range(NB)])
        im = {"xt": xt_t, "w1": w1s, "w2": w2s, "g": g_t}
        if has_b1:
            im["b1"] = np.ascontiguousarray(np.stack(
                [b1[labels[c][b]].reshape(2, MP, P) for b in range(NB)]
            ).transpose(3, 0, 1, 2))
        in_maps.append(im)

    res = run_bass_kernel_spmd(nc, in_maps, list(range(NCORES)))

    out = np.zeros((N, D), dtype=np.float32)
    for c in range(NCORES):
        y = res.results[c]["y"]
        t0 = 0
        for b in range(NB):
            tk, _ = slot_fill[(c, b)]
            if len(tk):
                np.add.at(out, tk, y[t0:t0 + len(tk)])
            t0 += TWs[b]

    if np.any(b2):
        comb = np.zeros((N, E), dtype=np.float32)
        comb[np.arange(N), idx1] += g1
        comb[np.arange(N), idx2] += g2
        out += comb @ b2
    return out.reshape(Bb, T, D)
